# revision 32
# baseline (speedup 1.0000x reference)
"""ClusterGCN + 2x GAT message-passing kernel for 8 Trainium2 NeuronCores.

Slot-aligned strategy (v2):
  - Nodes are sorted by in-degree (self-loops included) and dealt round-robin
    into 98 tiles x 128 slots per core, so all 8 cores see the same
    compile-time degree schedule.  Message k of dst node (tile j, slot p)
    lives at partition p, column k: no selection matmuls are needed — the
    segment reduction is a per-partition add-tree on the Vector engine.
  - Tiles are grouped into ~15 batches with uniform per-batch max degree Db.
    Layer-1 messages depend only on x, so they are materialized host-side
    (xedge) and streamed with plain contiguous DMAs -- no indirection at
    all.  Layers 2/3 gather z-rows with one [P,1]-index indirect DMA per
    message column (the HW dma_indirect1d uCode consumes exactly one index
    per partition per call).  Padded slots point at an all-zero table row.
  - GAT attention: s_src rides in the gathered row (bf16 hi/lo pair);
    s_dst is per-dst-node == per-partition, so it broadcasts for free.
    exp runs without max-subtraction (logits are small).  A host-built 0/1
    mask kills padded slots in the softmax denominator.
  - Between layers the per-core packed z-tables are AllGathered so any core
    can gather any source row.
"""

import sys

sys.path.insert(0, "/opt/trn_rl_repo")

import numpy as np

import concourse.bacc as bacc
import concourse.bass as bass
import concourse.mybir as mybir
import concourse.tile as tile
from concourse.bass_utils import run_bass_kernel_spmd

# ---- problem constants (hardcoded per contest rules) ----
N = 100000
E = 1600000
FIN = 64
HID = 64
FOUT = 32
NEG = 0.2

P = 128
NCORES = 8
TPC = 98                  # tiles per core
NPC = TPC * P             # 12544 nodes per core
BLK = NPC + P             # + one 128-row zero block per core
NROWS = NCORES * BLK      # gather-table rows
NP_ALL = NCORES * NPC

FW1 = 66                  # z1 row: z(64) | s_hi | s_lo
FW2 = 34                  # z2 row: z(32) | s_hi | s_lo
COLS_MAX = 128            # max message columns per compute batch
TB_MAX = 16               # max tiles per batch
GMAX = 1                  # one index per partition per indirect DMA (HW limit)

CF = 555

F32 = mybir.dt.float32
BF16 = mybir.dt.bfloat16
I32 = mybir.dt.int32
AF = mybir.ActivationFunctionType
OP = mybir.AluOpType
AX = mybir.AxisListType

_cache = {}
last_result = None


def _bf16(a):
    import ml_dtypes

    return np.asarray(a, dtype=ml_dtypes.bfloat16)


# ----------------------------------------------------------------------------
# host-side preprocessing
# ----------------------------------------------------------------------------
def _preprocess(x, edge_index):
    src = np.asarray(edge_index[0], np.int64)
    dst = np.asarray(edge_index[1], np.int64)
    loops = np.arange(N, dtype=np.int64)
    src_all = np.concatenate([loops, src])
    dst_all = np.concatenate([loops, dst])
    deg = np.bincount(dst_all, minlength=NP_ALL)  # padded nodes have deg 0

    order = np.argsort(-deg, kind="stable")
    rank = np.empty(NP_ALL, np.int64)
    rank[order] = np.arange(NP_ALL)
    g = rank // P
    core = g % NCORES
    j = g // NCORES
    p = rank % P
    row_of_node = core * BLK + j * P + p
    orow_of_node = core * NPC + j * P + p

    degm = np.zeros((NCORES, TPC), np.int64)
    np.maximum.at(degm, (core, j), deg)
    D_global = np.maximum(degm.max(axis=0), 1)

    batches = []  # (j0, Tb, Db, off)
    off = 0
    j0 = 0
    while j0 < TPC:
        Db = int(D_global[j0])
        Tb = 1
        while j0 + Tb < TPC and Tb < TB_MAX and (Tb + 1) * Db <= COLS_MAX:
            Tb += 1
        batches.append((j0, Tb, Db, off))
        off += Tb * Db
        j0 += Tb
    ctot = off

    col0 = np.zeros(TPC, np.int64)
    for (jj0, Tb, Db, boff) in batches:
        for t in range(Tb):
            col0[jj0 + t] = boff + t * Db

    midx = np.empty((NCORES, P, ctot), np.int32)
    for c in range(NCORES):
        midx[c, :, :] = c * BLK + NPC  # zero row
    mask = np.zeros((NCORES, P, ctot), np.float32)

    o2 = np.argsort(dst_all, kind="stable")
    ds = dst_all[o2]
    ss = src_all[o2]
    starts = np.searchsorted(ds, np.arange(NP_ALL))
    k = np.arange(len(ds)) - starts[ds]
    midx[core[ds], p[ds], col0[j[ds]] + k] = row_of_node[ss]
    mask[core[ds], p[ds], col0[j[ds]] + k] = 1.0

    deg_inv = (1.0 / np.maximum(deg, 1.0)).astype(np.float32)
    dinv = np.zeros((NCORES, P, TPC), np.float32)
    dinv[core, p, j] = deg_inv

    xv = np.asarray(x, np.float32)
    xtab = np.zeros((NROWS, FIN), np.float32)
    xtab[row_of_node[:N]] = xv[:N]
    xlt = np.zeros((NCORES, FIN, NPC), np.float32)
    xlt[core[:N], :, (j * P + p)[:N]] = xv[:N]

    # layer-1 messages are static: materialize them host-side (bf16), so
    # layer 1 needs no indirection at all — pure contiguous streaming.
    import ml_dtypes
    xtab_b = xtab.astype(ml_dtypes.bfloat16)
    xedge = xtab_b[midx]  # [NCORES, P, ctot, FIN] bf16

    # snake-permute the gather indices for layers 2/3: the indirect1d uCode
    # reads the index tensor as a flat list wrapped partition-first, while
    # the destination AP walks partition-major.  Permute per <=GMAX-column
    # chunk so device chunking matches.
    midx_snake = np.empty_like(midx)
    for (j0, Tb, Db, boff) in batches:
        cols = Tb * Db
        for ch in range(0, cols, GMAX):
            cc = min(GMAX, cols - ch)
            blk = midx[:, :, boff + ch : boff + ch + cc]  # [C, P, cc]
            flat = blk.reshape(NCORES, P * cc)            # n = p*cc + c
            snake = np.transpose(
                flat.reshape(NCORES, cc, P), (0, 2, 1)
            )  # [C, P, cc]: snake[:, n%128, n//128] = flat[:, n]
            midx_snake[:, :, boff + ch : boff + ch + cc] = snake

    return dict(
        batches=tuple(batches), ctot=ctot, midx=midx, mask=mask,
        midx_snake=midx_snake, xedge=xedge,
        dinv=dinv, xtab=xtab, xlt=xlt, orow=orow_of_node,
    )


def _padP(a):
    out = np.zeros((P, a.shape[1]), a.dtype)
    out[: a.shape[0]] = a
    return out


def _hilo(v):
    hi = _bf16(np.asarray(v, np.float32))
    lo = _bf16(np.asarray(v, np.float32) - np.asarray(hi, np.float32))
    return np.float32(hi), np.float32(lo)


# ----------------------------------------------------------------------------
# device program
# ----------------------------------------------------------------------------
def _build_program(batches, ctot, debug=False):
    nc = bacc.Bacc()

    xedge = nc.declare_dram_parameter(
        "xedge", [P, ctot * FIN], BF16, isOutput=False
    )
    xlt = nc.declare_dram_parameter("xlt", [FIN, NPC], F32, isOutput=False)
    cf_in = nc.declare_dram_parameter("constf", [P, CF], F32, isOutput=False)
    ci_in = nc.declare_dram_parameter("consti", [P, ctot], I32, isOutput=False)
    cm_in = nc.declare_dram_parameter("constm", [P, ctot], BF16, isOutput=False)
    outloc = nc.declare_dram_parameter("outloc", [NPC, FOUT], F32, isOutput=True)
    dbg = {}
    if debug:
        dbg["sd1"] = nc.declare_dram_parameter("dbg_sd1", [P, TPC], F32, isOutput=True)
        dbg["sd2"] = nc.declare_dram_parameter("dbg_sd2", [P, TPC], F32, isOutput=True)
        dbg["s2"] = nc.declare_dram_parameter("dbg_s2", [P, ctot], F32, isOutput=True)
        dbg["w2"] = nc.declare_dram_parameter("dbg_w2", [P, ctot], F32, isOutput=True)
        dbg["h2"] = nc.declare_dram_parameter("dbg_h2", [NPC, HID], F32, isOutput=True)
        dbg["s3"] = nc.declare_dram_parameter("dbg_s3", [P, ctot], F32, isOutput=True)
        dbg["agg1"] = nc.declare_dram_parameter("dbg_agg1", [NPC, FIN], F32, isOutput=True)

    z1loc = nc.dram_tensor("z1loc", [BLK, FW1], BF16)
    z1tab = nc.dram_tensor("z1tab", [NROWS, FW1], BF16, addr_space="Shared")
    z2loc = nc.dram_tensor("z2loc", [BLK, FW2], BF16)
    z2tab = nc.dram_tensor("z2tab", [NROWS, FW2], BF16, addr_space="Shared")

    groups = [list(range(NCORES))]

    with tile.TileContext(nc) as tc:
        with (
            tc.tile_pool(name="const", bufs=1) as cpool,
            tc.tile_pool(name="sbuf", bufs=3) as pool,
            tc.tile_pool(name="big", bufs=2) as bpool,
            tc.tile_pool(name="gath", bufs=3) as gpool,
            tc.tile_pool(name="psum", bufs=1, space="PSUM") as ptp,
        ):
            def cload(ap, shape, dt, tag):
                t = cpool.tile(shape, dt, tag=tag, name=tag)
                nc.sync.dma_start(out=t[:], in_=ap)
                return t

            cf = cload(cf_in[:, :], [P, CF], F32, tag="cf")
            ci = cload(ci_in[:, :], [P, ctot], I32, tag="ci")
            cm = cload(cm_in[:, :], [P, ctot], BF16, tag="cm")
            ident_t = cf[:, 0:128]
            dinv_t = cf[:, 128:226]
            b1r_t = cf[:, 226:290]
            b2r_t = cf[:, 290:322]
            bout_t = cf[:HID, 322:323]
            a1_t = cf[:HID, 323:327]
            a2_t = cf[:FOUT, 327:331]
            wout_t = cf[:FIN, 331:395]
            wroot_t = cf[:FIN, 395:459]
            w1_t = cf[:HID, 459:523]
            w2_t = cf[:HID, 523:555]

            sdcol1 = cpool.tile([P, TPC], F32, tag="sd1", name="sdcol1")
            sdcol2 = cpool.tile([P, TPC], F32, tag="sd2", name="sdcol2")

            # zero the pad block of the z tables (pads gather from there)
            zp1 = pool.tile([P, FW1], BF16, tag="zp1", name="zp1")
            nc.vector.memset(zp1[:], 0.0)
            nc.sync.dma_start(out=z1loc[NPC:BLK, :], in_=zp1[:])
            zp2 = pool.tile([P, FW2], BF16, tag="zp2", name="zp2")
            nc.vector.memset(zp2[:], 0.0)
            nc.sync.dma_start(out=z2loc[NPC:BLK, :], in_=zp2[:])

            def gather(tab, fw, off, cols, Tb, Db, zself, j0):
                msg = gpool.tile([P, Tb, Db, fw], BF16, tag="gmsg", name="msg")
                for c in range(cols):
                    t, dd = c // Db, c % Db
                    if dd == 0:
                        # k=0 is every real node's self-loop: its own local
                        # z-row, contiguous in zself -- no indirection.
                        jj = j0 + t
                        nc.sync.dma_start(
                            out=msg[:, t, 0, :],
                            in_=zself[jj * P : (jj + 1) * P, 0:fw],
                        )
                        continue
                    nc.gpsimd.indirect_dma_start(
                        out=msg[:, t, dd, :],
                        out_offset=None,
                        in_=tab[:, :],
                        in_offset=bass.IndirectOffsetOnAxis(
                            ap=ci[:, off + c : off + c + 1], axis=0
                        ),
                    )
                return msg

            def tree_reduce(msg4, Tb, Db, F):
                """Sum [P,Tb,Db,F] (bf16) over axis 2 -> f32 at [:, :, 0, :]."""
                H = (Db + 1) // 2
                red = bpool.tile([P, Tb, H, F], F32, tag="red", name="red")
                h0 = Db // 2
                if h0 > 0:
                    nc.vector.tensor_tensor(
                        out=red[:, :, 0:h0, :],
                        in0=msg4[:, :, 0:h0, :],
                        in1=msg4[:, :, h0 : 2 * h0, :],
                        op=OP.add,
                    )
                if Db % 2 == 1:
                    nc.vector.tensor_copy(
                        out=red[:, :, H - 1 : H, :], in_=msg4[:, :, 2 * h0 : Db, :]
                    )
                cur = H
                while cur > 1:
                    h = cur // 2
                    nc.vector.tensor_tensor(
                        out=red[:, :, 0:h, :],
                        in0=red[:, :, 0:h, :],
                        in1=red[:, :, h : 2 * h, :],
                        op=OP.add,
                    )
                    if cur % 2 == 1:
                        nc.vector.tensor_tensor(
                            out=red[:, :, 0:1, :],
                            in0=red[:, :, 0:1, :],
                            in1=red[:, :, 2 * h : cur, :],
                            op=OP.add,
                        )
                    cur = h
                return red

            def transform_pack(hT_sb, w_t, a_t, fo, zrow_b, t, jj, sdcol_t):
                """hT_sb [fi, P] f32 -> z=h@W packed (bf16 z | s_hi | s_lo)
                into zrow_b[:, t, :]; stash s_dst into sdcol_t[:, jj]."""
                zT_ps = ptp.tile([fo, P], F32, tag="zT", name="zT_ps")
                nc.tensor.matmul(
                    out=zT_ps[:], lhsT=w_t, rhs=hT_sb[:, :], start=True, stop=True
                )
                zT_sb = pool.tile([fo, P], F32, tag="zTsb", name="zT_sb")
                nc.vector.tensor_copy(out=zT_sb[:], in_=zT_ps[:])
                sc_ps = ptp.tile([P, 4], F32, tag="sc", name="sc_ps")
                nc.tensor.matmul(
                    out=sc_ps[:], lhsT=zT_sb[:, :], rhs=a_t, start=True, stop=True
                )
                sc_sb = pool.tile([P, 4], F32, tag="scsb", name="sc_sb")
                nc.vector.tensor_copy(out=sc_sb[:], in_=sc_ps[:, :])
                ssrc = pool.tile([P, 1], F32, tag="ssrc", name="ssrc")
                nc.vector.tensor_tensor(
                    out=ssrc[:], in0=sc_sb[:, 0:1], in1=sc_sb[:, 1:2], op=OP.add
                )
                nc.vector.tensor_tensor(
                    out=sdcol_t[:, jj : jj + 1],
                    in0=sc_sb[:, 2:3],
                    in1=sc_sb[:, 3:4],
                    op=OP.add,
                )
                zr_ps = ptp.tile([P, fo], F32, tag="zr", name="zr_ps")
                nc.tensor.transpose(
                    out=zr_ps[:], in_=zT_sb[:, :], identity=ident_t[:fo, 0:fo]
                )
                nc.vector.tensor_copy(out=zrow_b[:, t, 0:fo], in_=zr_ps[:, :])
                nc.vector.tensor_copy(
                    out=zrow_b[:, t, fo : fo + 1], in_=ssrc[:, :]
                )
                shi_f = pool.tile([P, 1], F32, tag="shif", name="shi_f")
                nc.vector.tensor_copy(
                    out=shi_f[:], in_=zrow_b[:, t, fo : fo + 1]
                )
                nc.vector.tensor_tensor(
                    out=zrow_b[:, t, fo + 1 : fo + 2],
                    in0=ssrc[:, :],
                    in1=shi_f[:, :],
                    op=OP.subtract,
                )

            # ================= Layer 1: ClusterGCN =================
            for (j0, Tb, Db, off) in batches:
                cols = Tb * Db
                msg = gpool.tile([P, Tb, Db, FIN], BF16, tag="gmsg", bufs=4, name="msg")
                nc.sync.dma_start(
                    out=msg.rearrange("p a b f -> p (a b f)"),
                    in_=xedge[:, off * FIN : (off + cols) * FIN],
                )
                red = tree_reduce(msg, Tb, Db, FIN)
                nc.vector.tensor_tensor(
                    out=red[:, :, 0, :],
                    in0=red[:, :, 0, :],
                    in1=dinv_t[:, j0 : j0 + Tb, None].to_broadcast([P, Tb, FIN]),
                    op=OP.mult,
                )
                if debug:
                    nc.sync.dma_start(
                        out=dbg["agg1"][j0 * P : (j0 + Tb) * P, :].rearrange(
                            "(t p) f -> p t f", p=P
                        ),
                        in_=red[:, :, 0, :],
                    )
                xTb = pool.tile([FIN, Tb * P], F32, tag="xTb", name="xTb")
                nc.sync.dma_start(
                    out=xTb[:, :], in_=xlt[:, j0 * P : (j0 + Tb) * P]
                )
                zrow_b = pool.tile([P, Tb, FW1], BF16, tag="zrow1", name="zrow_b")
                for t in range(Tb):
                    jj = j0 + t
                    aT_ps = ptp.tile([FIN, P], F32, tag="tp", bufs=2, name="aT_ps")
                    nc.tensor.transpose(
                        out=aT_ps[:], in_=red[:, t, 0, :], identity=ident_t
                    )
                    aT_sb = pool.tile([FIN, P], F32, tag="aT", name="aT_sb")
                    nc.vector.tensor_copy(out=aT_sb[:], in_=aT_ps[:])
                    hT_ps = ptp.tile([HID, P], F32, tag="hT", bufs=2, name="hT_ps")
                    nc.tensor.matmul(
                        out=hT_ps[:], lhsT=wout_t, rhs=aT_sb[:, :],
                        start=True, stop=False,
                    )
                    nc.tensor.matmul(
                        out=hT_ps[:], lhsT=wroot_t, rhs=xTb[:, t * P : (t + 1) * P],
                        start=False, stop=True,
                    )
                    h1T = pool.tile([HID, P], F32, tag="h1T", name="h1T")
                    nc.scalar.activation(
                        out=h1T[:], in_=hT_ps[:], func=AF.Relu, bias=bout_t
                    )
                    transform_pack(h1T, w1_t, a1_t, HID, zrow_b, t, jj, sdcol1)
                    nc.sync.dma_start(
                        out=z1loc[jj * P : (jj + 1) * P, :],
                        in_=zrow_b[:, t, :],
                    )

            if debug:
                nc.sync.dma_start(out=dbg["sd1"][:, :], in_=sdcol1[:])

            tc.strict_bb_all_engine_barrier()
            nc.gpsimd.collective_compute(
                "AllGather",
                OP.bypass,
                replica_groups=groups,
                ins=[z1loc[:, :]],
                outs=[z1tab[:, :]],
            )
            tc.strict_bb_all_engine_barrier()

            # ================= Layers 2 & 3: GAT =================
            def gat_layer(tab, zself, fw, fz, sdprev, w_t, a_t, brow_t,
                          zloc_next, fo_next, sdnext, last, dbg_s=None,
                          dbg_w=None, dbg_h=None):
                for (j0, Tb, Db, off) in batches:
                    cols = Tb * Db
                    msg = gather(tab, fw, off, cols, Tb, Db, zself, j0)
                    s = pool.tile([P, Tb, Db], F32, tag="s", name="s")
                    nc.vector.tensor_reduce(
                        out=s[:, :, :], in_=msg[:, :, :, fz : fz + 2],
                        axis=AX.X, op=OP.add,
                    )
                    nc.vector.tensor_tensor(
                        out=s[:, :, :],
                        in0=s[:, :, :],
                        in1=sdprev[:, j0 : j0 + Tb, None].to_broadcast(
                            [P, Tb, Db]
                        ),
                        op=OP.add,
                    )
                    if dbg_s is not None:
                        nc.sync.dma_start(
                            out=dbg_s[:, off : off + cols],
                            in_=s.rearrange("p a b -> p (a b)"),
                        )
                    lr = pool.tile([P, Tb, Db], F32, tag="lr", name="lr")
                    nc.vector.tensor_scalar(
                        out=lr[:], in0=s[:, :, :], scalar1=NEG, scalar2=None,
                        op0=OP.mult,
                    )
                    nc.vector.tensor_tensor(
                        out=lr[:], in0=s[:, :, :], in1=lr[:], op=OP.max
                    )
                    w = pool.tile([P, Tb, Db], F32, tag="w", name="w")
                    nc.scalar.activation(out=w[:], in_=lr[:], func=AF.Exp)
                    wm = pool.tile([P, Tb, Db], F32, tag="wm", name="wm")
                    nc.vector.tensor_tensor(
                        out=wm[:, :, :],
                        in0=w[:, :, :],
                        in1=cm[:, off : off + cols].rearrange(
                            "q (a b) -> q a b", a=Tb
                        ),
                        op=OP.mult,
                    )
                    if dbg_w is not None:
                        nc.sync.dma_start(
                            out=dbg_w[:, off : off + cols],
                            in_=wm.rearrange("p a b -> p (a b)"),
                        )
                    den = pool.tile([P, Tb], F32, tag="den", name="den")
                    nc.vector.tensor_reduce(
                        out=den[:, :], in_=wm[:, :, :], axis=AX.X, op=OP.add
                    )
                    nc.vector.tensor_scalar(
                        out=den[:], in0=den[:], scalar1=1e-30, scalar2=None,
                        op0=OP.max,
                    )
                    rec = pool.tile([P, Tb], F32, tag="rec", name="rec")
                    nc.vector.reciprocal(out=rec[:], in_=den[:])
                    wb = pool.tile([P, Tb, Db], BF16, tag="wb", name="wb")
                    nc.vector.tensor_copy(out=wb[:], in_=w[:, :, :])
                    mp = bpool.tile([P, Tb, Db, fz], BF16, tag="mp", name="mp")
                    nc.vector.tensor_tensor(
                        out=mp[:, :, :, :],
                        in0=msg[:, :, :, 0:fz],
                        in1=wb[:, :, :, None].to_broadcast([P, Tb, Db, fz]),
                        op=OP.mult,
                    )
                    redg = tree_reduce(mp, Tb, Db, fz)
                    h = pool.tile([P, Tb, fz], F32, tag="h", name="h")
                    nc.vector.tensor_tensor(
                        out=h[:, :, :],
                        in0=redg[:, :, 0, :],
                        in1=rec[:, :, None].to_broadcast([P, Tb, fz]),
                        op=OP.mult,
                    )
                    nc.vector.tensor_tensor(
                        out=h[:, :, :],
                        in0=h[:, :, :],
                        in1=brow_t[:, None, 0:fz].to_broadcast([P, Tb, fz]),
                        op=OP.add,
                    )
                    if dbg_h is not None:
                        nc.sync.dma_start(
                            out=dbg_h[j0 * P : (j0 + Tb) * P, 0:fz].rearrange(
                                "(t p) f -> p t f", p=P
                            ),
                            in_=h[:, :, :],
                        )
                    if last:
                        for t in range(Tb):
                            jj = j0 + t
                            nc.sync.dma_start(
                                out=outloc[jj * P : (jj + 1) * P, :],
                                in_=h[:, t, :],
                            )
                    else:
                        nc.vector.tensor_scalar(
                            out=h[:], in0=h[:, :, :], scalar1=0.0, scalar2=None,
                            op0=OP.max,
                        )
                        fw_next = fo_next + 2
                        zrow_b = pool.tile(
                            [P, Tb, fw_next], BF16, tag="zrow2", name="zrow_b2"
                        )
                        for t in range(Tb):
                            jj = j0 + t
                            hT_ps = ptp.tile(
                                [fz, P], F32, tag="tp", bufs=2, name="hT_ps2"
                            )
                            nc.tensor.transpose(
                                out=hT_ps[:], in_=h[:, t, :], identity=ident_t
                            )
                            hT_sb = pool.tile([fz, P], F32, tag="hTs", name="hT_sb")
                            nc.vector.tensor_copy(out=hT_sb[:], in_=hT_ps[:])
                            transform_pack(
                                hT_sb, w_t, a_t, fo_next, zrow_b, t, jj, sdnext
                            )
                            nc.sync.dma_start(
                                out=zloc_next[jj * P : (jj + 1) * P, :],
                                in_=zrow_b[:, t, :],
                            )

            gat_layer(z1tab, z1loc, FW1, HID, sdcol1, w2_t, a2_t, b1r_t,
                      z2loc, FOUT, sdcol2, False,
                      dbg_s=dbg.get("s2"), dbg_w=dbg.get("w2"),
                      dbg_h=dbg.get("h2"))
            if debug:
                nc.sync.dma_start(out=dbg["sd2"][:, :], in_=sdcol2[:])

            tc.strict_bb_all_engine_barrier()
            nc.gpsimd.collective_compute(
                "AllGather",
                OP.bypass,
                replica_groups=groups,
                ins=[z2loc[:, :]],
                outs=[z2tab[:, :]],
            )
            tc.strict_bb_all_engine_barrier()

            gat_layer(z2tab, z2loc, FW2, FOUT, sdcol2, None, None, b2r_t,
                      None, None, None, True, dbg_s=dbg.get("s3"))

    nc.finalize()
    return nc


# ----------------------------------------------------------------------------
# entry point
# ----------------------------------------------------------------------------
def kernel(
    x,
    edge_index,
    W_out,
    b_out,
    W_root,
    W1,
    a_src1,
    a_dst1,
    b1,
    W2,
    a_src2,
    a_dst2,
    b2,
    training=0,
    **_unused,
):
    import os
    debug = bool(os.environ.get("KERNEL_DEBUG"))
    pre = _preprocess(x, edge_index)
    key = (pre["batches"], pre["ctot"], debug)
    if key not in _cache:
        _cache[key] = _build_program(pre["batches"], pre["ctot"], debug=debug)
    nc = _cache[key]

    ident = np.eye(P, dtype=np.float32)
    a1hi, a1lo = _hilo(np.asarray(a_src1, np.float32))
    a1dhi, a1dlo = _hilo(np.asarray(a_dst1, np.float32))
    a2hi, a2lo = _hilo(np.asarray(a_src2, np.float32))
    a2dhi, a2dlo = _hilo(np.asarray(a_dst2, np.float32))
    a1 = np.stack([a1hi, a1lo, a1dhi, a1dlo], 1)
    a2 = np.stack([a2hi, a2lo, a2dhi, a2dlo], 1)

    in_maps = []
    for c in range(NCORES):
        in_maps.append(
            {
                "xedge": np.ascontiguousarray(
                    pre["xedge"][c].reshape(P, -1)
                ),
                "xlt": np.ascontiguousarray(pre["xlt"][c]),
                "constf": np.concatenate(
                    [
                        ident,
                        pre["dinv"][c],
                        np.tile(np.asarray(b1, np.float32), (P, 1)),
                        np.tile(np.asarray(b2, np.float32), (P, 1)),
                        _padP(np.asarray(b_out, np.float32).reshape(HID, 1)),
                        _padP(a1),
                        _padP(a2),
                        _padP(np.asarray(W_out, np.float32)),
                        _padP(np.asarray(W_root, np.float32)),
                        _padP(np.asarray(W1, np.float32)),
                        _padP(np.asarray(W2, np.float32)),
                    ],
                    axis=1,
                ),
                "consti": pre["midx_snake"][c],
                "constm": _bf16(pre["mask"][c]),
            }
        )

    import os
    trace = bool(os.environ.get("BASS_TRACE"))
    res = run_bass_kernel_spmd(nc, in_maps, list(range(NCORES)), trace=trace)
    global last_result, last_pre
    last_result = res
    last_pre = pre
    out_p = np.concatenate([res.results[c]["outloc"] for c in range(NCORES)], 0)
    out = out_p[pre["orow"][:N]]
    return np.asarray(out, np.float32)
